# revision 28
# baseline (speedup 1.0000x reference)
"""Trainium2 Bass kernel for nn_MultiHeadAttention_21345987461791 (v3).

Reference computation (B=4, S=4096, HID=1024, NUM_HEADS=16, HEAD_DIM=64):
    qh = (q @ Wq + bq)  -> [B,S,16,64]   (same for k, v)
    scores = einsum('bshd,bstd->bsht', qh, kh) / 8     # per-token [16,16]
    w = softmax(scores, -1)
    out = einsum('bsht,bstd->bshd', w, vh) @ Wo + bo
    LayerNorm-ish: ln_w * (out - mu) / (var_unbiased + eps) + ln_b   (NO sqrt!)

Sharding: pure data-parallel over tokens (2048/core, 8 cores, no collectives).

v3 design: the per-token [16,16] attention runs on the PE (tensor engine)
instead of the DVE:
  - host pre-transposes q/k/v to [in-feat, token] bf16, so projections run
    as out[feat-chunk, tok] = W_chunk^T @ xT_chunk with no on-chip input
    transposes;
  - scores: per-8-token-group cross matmuls over d (qS/kS regrouped to
    [64 d, head*128+s] via strided SBUF DMAs); diagonal [16,16] blocks are
    extracted from PSUM by a GPSIMD ap_gather;
  - softmax on ACT (exp) + DVE (sum/recip/mult);
  - attention-out: per-token k=16 matmuls lhsT=vS[16 t, d], rhs=wTT[16 t, h]
    (wgt transposed per-group on PE);
  - attn result [64 d, (h,s)] is re-laid out to [feat, tok] via PE
    transposes, then the o-projection + LayerNorm (stats via ACT accumulate
    + DVE tensor_scalar apply).
"""

import numpy as np
import ml_dtypes

import concourse.bass as bass
import concourse.bacc as bacc
import concourse.mybir as mybir
import concourse.tile as tile
from concourse.bass_utils import run_bass_kernel_spmd
from concourse.masks import make_identity

B, S, HID = 4, 4096, 1024
H, D = 16, 64
EPS = 1e-5
NCORES = 8
TOKENS = B * S
T_CORE = TOKENS // NCORES          # 2048
P = 128
NT = T_CORE // P                   # 16 token tiles per core
KC = HID // P                      # 8 contraction chunks
SCALE = 1.0 / (D ** 0.5)

F32 = mybir.dt.float32
BF16 = mybir.dt.bfloat16
I16 = mybir.dt.int16
AX = mybir.AxisListType
OP = mybir.AluOpType
ACTF = mybir.ActivationFunctionType

_CACHE = {}


def _bcast_dram(ap, p=P):
    """AP that reads a [N] dram vector replicated across p partitions."""
    return bass.AP(tensor=ap.tensor, offset=ap.offset, ap=[[0, p]] + list(ap.ap))


def _build_bass(nt=NT, reps=1, trivial=False):
    nc = bacc.Bacc("TRN2", target_bir_lowering=False, debug=False,
                   num_devices=NCORES)

    # host-pretransposed inputs [ki=128, kc=8, T_CORE] bf16
    q_d = nc.dram_tensor("q", [P, KC, T_CORE], BF16, kind="ExternalInput").ap()
    k_d = nc.dram_tensor("k", [P, KC, T_CORE], BF16, kind="ExternalInput").ap()
    v_d = nc.dram_tensor("v", [P, KC, T_CORE], BF16, kind="ExternalInput").ap()
    # weights pre-arranged on host to [ki=128, kc=8, n=1024] bf16
    wq_d = nc.dram_tensor("wq", [P, KC, HID], BF16, kind="ExternalInput").ap()
    wk_d = nc.dram_tensor("wk", [P, KC, HID], BF16, kind="ExternalInput").ap()
    wv_d = nc.dram_tensor("wv", [P, KC, HID], BF16, kind="ExternalInput").ap()
    wo_d = nc.dram_tensor("wo", [P, KC, HID], BF16, kind="ExternalInput").ap()
    # biases pre-arranged [128, 8] (partition-major per out-chunk)
    bq_d = nc.dram_tensor("bq", [P, KC], F32, kind="ExternalInput").ap()
    bk_d = nc.dram_tensor("bk", [P, KC], F32, kind="ExternalInput").ap()
    bv_d = nc.dram_tensor("bv", [P, KC], F32, kind="ExternalInput").ap()
    bo_d = nc.dram_tensor("bo", [HID], BF16, kind="ExternalInput").ap()
    lnw_d = nc.dram_tensor("lnw", [HID], BF16, kind="ExternalInput").ap()
    lnb_d = nc.dram_tensor("lnb", [HID], BF16, kind="ExternalInput").ap()
    gidx_d = nc.dram_tensor("gidx", [P, 8], I16, kind="ExternalInput").ap()
    out_d = nc.dram_tensor("out", [T_CORE, HID], BF16, kind="ExternalOutput").ap()

    with tile.TileContext(nc) as tc:
        _kernel_body(nc, tc, q_d, k_d, v_d, wq_d, wk_d, wv_d, wo_d,
                     bq_d, bk_d, bv_d, bo_d, lnw_d, lnb_d, gidx_d, out_d,
                     nt, reps, trivial)
    nc.compile()
    return nc


def _kernel_body(nc, tc, q_d, k_d, v_d, wq_d, wk_d, wv_d, wo_d,
                 bq_d, bk_d, bv_d, bo_d, lnw_d, lnb_d, gidx_d, out_d,
                 nt=NT, reps=1, trivial=False):
    from contextlib import ExitStack
    ctx = ExitStack()
    with ctx:
        singles = ctx.enter_context(tc.tile_pool(name="singles", bufs=1))
        x_pool = ctx.enter_context(tc.tile_pool(name="x", bufs=2))
        ph_pool = ctx.enter_context(tc.tile_pool(name="ph", bufs=2))
        rg_pool = ctx.enter_context(tc.tile_pool(name="rg", bufs=2))
        sm_pool = ctx.enter_context(tc.tile_pool(name="sm", bufs=2))
        at_pool = ctx.enter_context(tc.tile_pool(name="at", bufs=2))
        y_pool = ctx.enter_context(tc.tile_pool(name="y", bufs=2))
        scr_pool = ctx.enter_context(tc.tile_pool(name="scr", bufs=1))
        psum_p = ctx.enter_context(tc.tile_pool(name="psp", bufs=2, space="PSUM"))
        psum_x = ctx.enter_context(tc.tile_pool(name="psx", bufs=2, space="PSUM"))
        psum_a = ctx.enter_context(tc.tile_pool(name="psa", bufs=2, space="PSUM"))
        psum_nt = ctx.enter_context(tc.tile_pool(name="psnt", bufs=2, space="PSUM"))

        # ---- persistent constants ----
        # (x-tile DMAs for the first pair are issued first in stage_a; load
        # wq before wk/wv, and wo last since o-proj runs 3 stages later)
        w_sb = {}
        for name, wd in (("q", wq_d), ("k", wk_d), ("v", wv_d), ("o", wo_d)):
            wt = singles.tile([P, KC, HID], BF16, tag=f"w{name}")
            w_sb[name] = wt

        def load_weights():
            for name, wd in (("q", wq_d), ("k", wk_d), ("v", wv_d)):
                nc.sync.dma_start(w_sb[name][:], wd[:])
            nc.scalar.dma_start(w_sb["o"][:], wo_d[:])

        gidx = singles.tile([P, 8], I16, tag="gidx")
        nc.scalar.dma_start(gidx[:], gidx_d[:])
        identb = singles.tile([P, P], BF16, tag="identb")
        make_identity(nc, identb)

        if not trivial:
            b_sb = {}
            for name, bd in (("q", bq_d), ("k", bk_d), ("v", bv_d)):
                t = singles.tile([P, KC], F32, tag=f"b{name}")
                nc.sync.dma_start(t[:], bd[:])
                b_sb[name] = t
            bo_sb = singles.tile([P, HID], BF16, tag="bo")
            nc.sync.dma_start(bo_sb[:], _bcast_dram(bo_d))
            lnw_sb = singles.tile([P, HID], BF16, tag="lnw")
            nc.sync.dma_start(lnw_sb[:], _bcast_dram(lnw_d))
            lnb_sb = singles.tile([P, HID], BF16, tag="lnb")
            nc.sync.dma_start(lnb_sb[:], _bcast_dram(lnb_d))

        state = {}
        preloaded = {}

        def emit_x_pair(it):
            xs = {}
            for name, xd in (("q", q_d), ("k", k_d), ("v", v_d)):
                xt = x_pool.tile([P, KC, 2 * P], BF16, tag=f"x{name}")
                lo = it * P
                hi = min(lo + 2 * P, nt * P)
                nc.sync.dma_start(xt[:, :, 0:hi - lo], xd[:, :, lo:hi])
                xs[name] = xt
            return xs

        def stage_a(pos, it):
            """Load x pair + q/k/v projections -> qhT/khT/vhT chunk layout."""
            st = state[pos] = {}
            if pos % 2 == 0:
                st["x"] = preloaded.pop(pos, None) or emit_x_pair(it)
                off = 0
            else:
                st["x"] = state[pos - 1]["x"]
                off = P

            for name in ("q", "k", "v"):
                xt = st["x"][name]
                phT = ph_pool.tile([P, KC, P], BF16, tag=f"phT{name}")
                for bank in range(2):
                    pm = psum_p.tile([P, 512], F32, tag="proj")
                    for ocl in range(4):
                        oc = bank * 4 + ocl
                        for c in range(KC):
                            nc.tensor.matmul(
                                pm[:, ocl * P:(ocl + 1) * P],
                                lhsT=w_sb[name][:, c, oc * P:(oc + 1) * P],
                                rhs=xt[:, c, off:off + P],
                                start=(c == 0), stop=(c == KC - 1))
                    if trivial:
                        nc.scalar.activation(
                            out=phT[:, bank * 4:(bank + 1) * 4, :].rearrange(
                                "p a b -> p (a b)"),
                            in_=pm[:], func=ACTF.Copy)
                    else:
                        for ocl in range(4):
                            oc = bank * 4 + ocl
                            nc.scalar.activation(
                                out=phT[:, oc, :], in_=pm[:, ocl * P:(ocl + 1) * P],
                                func=ACTF.Identity, bias=b_sb[name][:, oc:oc + 1])
                st[f"phT{name}"] = phT

            # qS/kS [64 d, h*128+s] via 2 DMAs each (hl partition halves)
            qS = rg_pool.tile([64, 16 * P], BF16, tag="qS")
            kS = rg_pool.tile([64, 16 * P], BF16, tag="kS")
            for dst, src, eng in ((qS, st["phTq"], nc.sync),
                                  (kS, st["phTk"], nc.scalar)):
                for hl in range(2):
                    s_ap = src[hl * 64:(hl + 1) * 64, :, :]
                    d_ap = bass.AP(tensor=dst.tensor, offset=dst.offset + hl * P,
                                   ap=[[dst.ap[0][0], 64], [256, KC], [1, P]])
                    eng.dma_start(d_ap, s_ap)
            st["qS"], st["kS"] = qS, kS

        def stage_b1(pos):
            """Cross matmuls + gather + softmax + vS regroup for tile at `pos`."""
            st = state[pos]
            qS, kS, vhT = st["qS"], st["kS"], st["phTv"]

            # shuffle to group-contiguous layout [64, g*128 + si*16 + h]
            # (matmul operands allow only one free dim)
            qS2 = rg_pool.tile([64, 16 * P], BF16, tag="qS2")
            kS2 = rg_pool.tile([64, 16 * P], BF16, tag="kS2")
            for dst, src in ((qS2, qS), (kS2, kS)):
                s_ap = bass.AP(tensor=src.tensor, offset=src.offset,
                               ap=[[src.ap[0][0], 64], [8, 16], [1, 8], [P, 16]])
                nc.vector.tensor_copy(out=dst[:], in_=s_ap)

            # cross matmuls: 16 groups of 8 tokens; quarter-psum [128, 512];
            # exp applied during PSUM->SBUF eviction (ACT), then one GPSIMD
            # gather extracts the diagonal blocks (GPSIMD can't read PSUM).
            crossE = scr_pool.tile([P, 4, 512], BF16, tag="crossE")
            for quarter in range(4):
                pt = psum_x.tile([P, 512], F32, tag="cross")
                for gl in range(4):
                    g = quarter * 4 + gl
                    nc.tensor.matmul(pt[:, gl * P:(gl + 1) * P],
                                     lhsT=qS2[:, g * P:(g + 1) * P],
                                     rhs=kS2[:, g * P:(g + 1) * P],
                                     start=True, stop=True)
                nc.scalar.activation(out=crossE[:, quarter, :], in_=pt[:],
                                     func=ACTF.Exp)

            # gather exp'd diagonal: wgtE [128 (si,h), (g,t)]
            wgtE = sm_pool.tile([P, 256], BF16, tag="wgtE")
            cflat = crossE.rearrange("p a b -> p (a b)")
            nc.gpsimd.ap_gather(
                out_ap=wgtE.rearrange("p (j d) -> p j d", d=2),
                in_ap=cflat.rearrange("p (j d) -> p j d", d=2),
                idxs_ap=gidx[:], channels=P, num_elems=1024, d=2, num_idxs=128)
            ssum = sm_pool.tile([P, 16], F32, tag="ssum")
            nc.vector.tensor_reduce(
                out=ssum[:], in_=wgtE.rearrange("p (g t) -> p g t", g=16),
                axis=AX.X, op=OP.add)
            rs = sm_pool.tile([P, 16], F32, tag="rs")
            nc.vector.reciprocal(out=rs[:], in_=ssum[:])
            wgt = sm_pool.tile([P, 16, 16], BF16, tag="wgt")
            nc.vector.tensor_tensor(
                out=wgt[:], in0=wgtE.rearrange("p (g t) -> p g t", g=16),
                in1=rs[:, :, None].to_broadcast((P, 16, 16)), op=OP.mult)
            st["wgt"] = wgt

            # vS [16 t, d*128+s] via 8 DMAs (dl 16-partition slices)
            vS = rg_pool.tile([16, 64 * P], BF16, tag="vS")
            for dl in range(8):
                s_ap = vhT[dl * 16:(dl + 1) * 16, :, :]
                d_ap = bass.AP(tensor=vS.tensor, offset=vS.offset + dl * P,
                               ap=[[vS.ap[0][0], 16], [8 * P, KC], [1, P]])
                eng = nc.gpsimd if dl < 6 else nc.sync
                eng.dma_start(d_ap, s_ap)
            st["vS"] = vS

        def stage_b2a(pos):
            """wgt transpose, attn matmuls, attnT regroup."""
            st = state[pos]
            wgt, vS = st["wgt"], st["vS"]
            wgtf = wgt.rearrange("p g t -> p (g t)")
            # per-g transposes -> wTT [16 t, g, (si,h)]
            wTT = at_pool.tile([16, 16, P], BF16, tag="wTT")
            pw = psum_nt.tile([P, 8 * P], BF16, tag="nt")
            for g in range(8):
                nc.tensor.transpose(pw[0:16, g * P:(g + 1) * P],
                                    wgtf[:, g * 16:(g + 1) * 16], identb)
            nc.vector.tensor_copy(
                out=wTT[:, 0:8, :].rearrange("p a b -> p (a b)"), in_=pw[0:16, :])
            pw2 = psum_nt.tile([P, 8 * P], BF16, tag="nt")
            for g in range(8):
                nc.tensor.transpose(pw2[0:16, g * P:(g + 1) * P],
                                    wgtf[:, (g + 8) * 16:(g + 9) * 16], identb)
            nc.vector.tensor_copy(
                out=wTT[:, 8:16, :].rearrange("p a b -> p (a b)"), in_=pw2[0:16, :])

            # per-token attn matmuls -> psum [64 d, (s,h)] quarters
            attnSB = at_pool.tile([64, 16, P], BF16, tag="attnSB")
            for quarter in range(4):
                pa = psum_a.tile([64, 512], F32, tag="attnp")
                for sl in range(32):
                    s = quarter * 32 + sl
                    g, si = s // 8, s % 8
                    lhsT = bass.AP(tensor=vS.tensor, offset=vS.offset + s,
                                   ap=[[vS.ap[0][0], 16], [P, 64]])
                    rhs = wTT[:, g, si * 16:(si + 1) * 16]
                    nc.tensor.matmul(pa[:, sl * 16:(sl + 1) * 16], lhsT=lhsT,
                                     rhs=rhs, start=True, stop=True)
                # evict rearranged (sl,h) -> cols h*128 + (quarter*32+sl)
                d_ap = bass.AP(tensor=attnSB.tensor,
                               offset=attnSB.offset + quarter * 32,
                               ap=[[attnSB.ap[0][0], 64], [1, 32], [P, 16]])
                nc.vector.tensor_copy(out=d_ap, in_=pa[:])

            # attnT [feat=(h,d), s] chunk layout via 2 regroup DMAs
            # (mirror of the qS regroup: attnSB is [64 d, h*128+s])
            attnT = at_pool.tile([P, KC, P], BF16, tag="attnT")
            for hl in range(2):
                s_ap = bass.AP(tensor=attnSB.tensor,
                               offset=attnSB.offset + hl * P,
                               ap=[[attnSB.ap[0][0], 64], [256, KC], [1, P]])
                d_ap = attnT[hl * 64:(hl + 1) * 64, :, :]
                nc.sync.dma_start(d_ap, s_ap)
            st["attnT"] = attnT

        def stage_b2b(pos, it):
            """o-proj + LN + store for tile `pos`."""
            st = state[pos]
            attnT = st["attnT"]
            # o-projection: y[s, n] in 2 halves + LN stats via ACT accumulate
            y = y_pool.tile([P, HID], BF16, tag="y")
            ysum = y_pool.tile([P, 2], F32, tag="ysum")
            for half in range(2):
                pm = psum_p.tile([P, 512], F32, tag="proj")
                for c in range(KC):
                    nc.tensor.matmul(pm[:], lhsT=attnT[:, c, :],
                                     rhs=w_sb["o"][:, c, half * 512:(half + 1) * 512],
                                     start=(c == 0), stop=(c == KC - 1))
                if trivial:
                    nc.scalar.activation(
                        out=y[:, half * 512:(half + 1) * 512], in_=pm[:],
                        func=ACTF.Copy, accum_out=ysum[:, half:half + 1])
                else:
                    nc.vector.tensor_tensor(
                        out=y[:, half * 512:(half + 1) * 512], in0=pm[:],
                        in1=bo_sb[:, half * 512:(half + 1) * 512], op=OP.add)
                    nc.scalar.activation(
                        out=y[:, half * 512:(half + 1) * 512],
                        in_=y[:, half * 512:(half + 1) * 512],
                        func=ACTF.Copy, accum_out=ysum[:, half:half + 1])

            # LN-ish: yout = ln_w*(y-mu)/(var_ddof1+eps) + ln_b
            ysq2 = y_pool.tile([P, 2], F32, tag="ysq2")
            scr = scr_pool.tile([P, 512], BF16, tag="scr")
            for half in range(2):
                nc.scalar.activation(
                    out=scr[:], in_=y[:, half * 512:(half + 1) * 512],
                    func=ACTF.Square, accum_out=ysq2[:, half:half + 1])
            ysq = y_pool.tile([P, 1], F32, tag="ysq")
            nc.vector.tensor_tensor(out=ysq[:], in0=ysq2[:, 0:1],
                                    in1=ysq2[:, 1:2], op=OP.add)
            s1 = y_pool.tile([P, 1], F32, tag="s1")
            nc.vector.tensor_tensor(out=s1[:], in0=ysum[:, 0:1],
                                    in1=ysum[:, 1:2], op=OP.add)
            mu = y_pool.tile([P, 1], F32, tag="mu")
            nc.vector.tensor_scalar(out=mu[:], in0=s1[:], scalar1=1.0 / HID,
                                    scalar2=None, op0=OP.mult)
            sq = y_pool.tile([P, 1], F32, tag="sq")
            nc.vector.tensor_tensor(out=sq[:], in0=s1[:], in1=mu[:], op=OP.mult)
            varr = y_pool.tile([P, 1], F32, tag="varr")
            nc.vector.tensor_scalar(out=varr[:], in0=ysq[:], scalar1=sq[:],
                                    scalar2=1.0 / (HID - 1), op0=OP.subtract,
                                    op1=OP.mult)
            nc.vector.tensor_scalar(out=varr[:], in0=varr[:], scalar1=EPS,
                                    scalar2=None, op0=OP.add)
            rv = y_pool.tile([P, 1], F32, tag="rv")
            nc.vector.reciprocal(out=rv[:], in_=varr[:])
            # yout pair tile: even pos allocates, odd pos fills 2nd half,
            # one DMA per pair
            if pos % 2 == 0:
                youtp = y_pool.tile([P, 2, HID], BF16, tag="yout")
                st["youtp"] = youtp
            else:
                youtp = state[pos - 1]["youtp"]
            yv = youtp[:, pos % 2, :]
            nc.vector.tensor_scalar(out=yv, in0=y[:], scalar1=mu[:],
                                    scalar2=rv[:], op0=OP.subtract, op1=OP.mult)
            if not trivial:
                nc.vector.tensor_tensor(out=yv, in0=yv, in1=lnw_sb[:],
                                        op=OP.mult)
                nc.vector.tensor_tensor(out=yv, in0=yv, in1=lnb_sb[:],
                                        op=OP.add)
            if pos % 2 == 1:
                it0 = it - 1
                d_ap = bass.AP(tensor=out_d.tensor,
                               offset=out_d.offset + it0 * P * HID,
                               ap=[[HID, P], [P * HID, 2], [1, HID]])
                nc.sync.dma_start(d_ap, youtp[:, 0:2, :])
                del state[pos - 1], state[pos]

        # software pipeline: B1(i-1) | A(i) | B2a(i-2) | B2b(i-3)
        tiles = [t for _ in range(reps) for t in range(nt)]
        n = len(tiles)
        preloaded[0] = emit_x_pair(tiles[0])
        load_weights()
        for i, it in enumerate(tiles):
            if i >= 1:
                stage_b1(i - 1)
            stage_a(i, it)
            if i >= 2:
                stage_b2a(i - 2)
            if i >= 3:
                stage_b2b(i - 3, tiles[i - 3])
        stage_b1(n - 1)
        for j in (n - 2, n - 1):
            if j >= 0:
                stage_b2a(j)
        for j in (n - 3, n - 2, n - 1):
            if j >= 0:
                stage_b2b(j, tiles[j])


def get_nc(nt=NT, reps=1, trivial=False):
    key = f"nc{nt}_{reps}_{trivial}"
    if key not in _CACHE:
        _CACHE[key] = _build_bass(nt, reps, trivial)
    return _CACHE[key]


def _prep_host(Wq, bq, Wk, bk, Wv, bv, Wo, bo, ln_w, ln_b):
    """Host-side prep: fold score scale into Wq/bq; permute Wv cols to
    d-major; cast to bf16 and rearrange [K,N] -> [ki, kc, n]."""
    def rearr(w):
        return np.ascontiguousarray(
            np.asarray(w, dtype=np.float32).reshape(KC, P, HID).transpose(1, 0, 2)
        ).astype(ml_dtypes.bfloat16)

    # v out-feature permutation: col t*64+d -> pos d*16+t
    perm_v = np.argsort(
        np.arange(HID).reshape(H, D).T.reshape(-1), kind="stable")
    perm_v = np.arange(HID).reshape(H, D).T.reshape(-1)  # pos (d,t) <- t*64+d
    Wv_p = np.asarray(Wv, np.float32)[:, perm_v]
    bv_p = np.asarray(bv, np.float32)[perm_v]

    def bias_arr(b):
        return np.ascontiguousarray(
            np.asarray(b, np.float32).reshape(KC, P).T).astype(np.float32)

    gidx = np.zeros((P, 8), np.int16)
    for si in range(8):
        vals = np.array([g * 64 + si * 8 + t2 for g in range(16)
                         for t2 in range(8)], np.int16)
        gidx[si * 16:(si + 1) * 16, :] = vals.reshape(8, 16).T

    return {
        "wq": rearr(np.asarray(Wq, np.float32) * SCALE),
        "wk": rearr(Wk),
        "wv": rearr(Wv_p),
        "wo": rearr(Wo),
        "bq": bias_arr(np.asarray(bq, np.float32) * np.float32(SCALE)),
        "bk": bias_arr(bk),
        "bv": bias_arr(bv_p),
        "bo": np.asarray(bo, np.float32).astype(ml_dtypes.bfloat16),
        "lnw": np.asarray(ln_w, np.float32).astype(ml_dtypes.bfloat16),
        "lnb": np.asarray(ln_b, np.float32).astype(ml_dtypes.bfloat16),
        "gidx": gidx,
    }


def _prep_x(x):
    """[T_CORE, HID] f32 -> [128 ki, 8 kc, T_CORE] bf16 (pre-transposed)."""
    xT = np.asarray(x, np.float32).T  # [HID, T_CORE]
    return np.ascontiguousarray(
        xT.reshape(KC, P, T_CORE).transpose(1, 0, 2)).astype(ml_dtypes.bfloat16)


def make_in_maps(q, k, v, Wq, bq, Wk, bk, Wv, bv, Wo, bo, ln_w, ln_b):
    shared = _prep_host(Wq, bq, Wk, bk, Wv, bv, Wo, bo, ln_w, ln_b)
    qf = np.asarray(q, np.float32).reshape(TOKENS, HID)
    kf = np.asarray(k, np.float32).reshape(TOKENS, HID)
    vf = np.asarray(v, np.float32).reshape(TOKENS, HID)
    in_maps = []
    for c in range(NCORES):
        sl = slice(c * T_CORE, (c + 1) * T_CORE)
        m = dict(shared)
        m["q"] = _prep_x(qf[sl])
        m["k"] = _prep_x(kf[sl])
        m["v"] = _prep_x(vf[sl])
        in_maps.append(m)
    return in_maps


def kernel(q, k, v, Wq, bq, Wk, bk, Wv, bv, Wo, bo, ln_w, ln_b):
    trivial = (not np.any(np.asarray(bq)) and not np.any(np.asarray(bk))
               and not np.any(np.asarray(bv)) and not np.any(np.asarray(bo))
               and np.all(np.asarray(ln_w) == 1.0)
               and not np.any(np.asarray(ln_b)))
    nc = get_nc(trivial=trivial)
    in_maps = make_in_maps(q, k, v, Wq, bq, Wk, bk, Wv, bv, Wo, bo, ln_w, ln_b)
    res = run_bass_kernel_spmd(nc, in_maps, list(range(NCORES))).results
    out = np.concatenate([np.asarray(res[c]["out"]) for c in range(NCORES)],
                         axis=0)
    return out.reshape(B, S, HID).astype(np.float32)


# revision 29
# speedup vs baseline: 1.5278x; 1.5278x over previous
"""Trainium2 Bass kernel for nn_MultiHeadAttention_21345987461791 (v3).

Reference computation (B=4, S=4096, HID=1024, NUM_HEADS=16, HEAD_DIM=64):
    qh = (q @ Wq + bq)  -> [B,S,16,64]   (same for k, v)
    scores = einsum('bshd,bstd->bsht', qh, kh) / 8     # per-token [16,16]
    w = softmax(scores, -1)
    out = einsum('bsht,bstd->bshd', w, vh) @ Wo + bo
    LayerNorm-ish: ln_w * (out - mu) / (var_unbiased + eps) + ln_b   (NO sqrt!)

Sharding: pure data-parallel over tokens (2048/core, 8 cores, no collectives).

v3 design: the per-token [16,16] attention runs on the PE (tensor engine)
instead of the DVE:
  - host pre-transposes q/k/v to [in-feat, token] bf16, so projections run
    as out[feat-chunk, tok] = W_chunk^T @ xT_chunk with no on-chip input
    transposes;
  - scores: per-8-token-group cross matmuls over d (qS/kS regrouped to
    [64 d, head*128+s] via strided SBUF DMAs); diagonal [16,16] blocks are
    extracted from PSUM by a GPSIMD ap_gather;
  - softmax on ACT (exp) + DVE (sum/recip/mult);
  - attention-out: per-token k=16 matmuls lhsT=vS[16 t, d], rhs=wTT[16 t, h]
    (wgt transposed per-group on PE);
  - attn result [64 d, (h,s)] is re-laid out to [feat, tok] via PE
    transposes, then the o-projection + LayerNorm (stats via ACT accumulate
    + DVE tensor_scalar apply).
"""

import numpy as np
import ml_dtypes

import concourse.bass as bass
import concourse.bacc as bacc
import concourse.mybir as mybir
import concourse.tile as tile
from concourse.bass_utils import run_bass_kernel_spmd
from concourse.masks import make_identity

B, S, HID = 4, 4096, 1024
H, D = 16, 64
EPS = 1e-5
NCORES = 8
TOKENS = B * S
T_CORE = TOKENS // NCORES          # 2048
P = 128
NT = T_CORE // P                   # 16 token tiles per core
KC = HID // P                      # 8 contraction chunks
SCALE = 1.0 / (D ** 0.5)

F32 = mybir.dt.float32
BF16 = mybir.dt.bfloat16
I16 = mybir.dt.int16
AX = mybir.AxisListType
OP = mybir.AluOpType
ACTF = mybir.ActivationFunctionType

_CACHE = {}


def _bcast_dram(ap, p=P):
    """AP that reads a [N] dram vector replicated across p partitions."""
    return bass.AP(tensor=ap.tensor, offset=ap.offset, ap=[[0, p]] + list(ap.ap))


def _build_bass(nt=NT, reps=1, trivial=False):
    nc = bacc.Bacc("TRN2", target_bir_lowering=False, debug=False,
                   num_devices=NCORES)

    # host-pretransposed inputs [ki=128, kc=8, T_CORE] bf16
    q_d = nc.dram_tensor("q", [P, KC, T_CORE], BF16, kind="ExternalInput").ap()
    k_d = nc.dram_tensor("k", [P, KC, T_CORE], BF16, kind="ExternalInput").ap()
    v_d = nc.dram_tensor("v", [P, KC, T_CORE], BF16, kind="ExternalInput").ap()
    # weights pre-arranged on host to [ki=128, kc=8, n=1024] bf16
    wq_d = nc.dram_tensor("wq", [P, KC, HID], BF16, kind="ExternalInput").ap()
    wk_d = nc.dram_tensor("wk", [P, KC, HID], BF16, kind="ExternalInput").ap()
    wv_d = nc.dram_tensor("wv", [P, KC, HID], BF16, kind="ExternalInput").ap()
    wo_d = nc.dram_tensor("wo", [P, KC, HID], BF16, kind="ExternalInput").ap()
    # biases pre-arranged [128, 8] (partition-major per out-chunk)
    bq_d = nc.dram_tensor("bq", [P, KC], F32, kind="ExternalInput").ap()
    bk_d = nc.dram_tensor("bk", [P, KC], F32, kind="ExternalInput").ap()
    bv_d = nc.dram_tensor("bv", [P, KC], F32, kind="ExternalInput").ap()
    bo_d = nc.dram_tensor("bo", [HID], BF16, kind="ExternalInput").ap()
    lnw_d = nc.dram_tensor("lnw", [HID], BF16, kind="ExternalInput").ap()
    lnb_d = nc.dram_tensor("lnb", [HID], BF16, kind="ExternalInput").ap()
    gidx_d = nc.dram_tensor("gidx", [P, 8], I16, kind="ExternalInput").ap()
    out_d = nc.dram_tensor("out", [T_CORE, HID], BF16, kind="ExternalOutput").ap()

    with tile.TileContext(nc) as tc:
        _kernel_body(nc, tc, q_d, k_d, v_d, wq_d, wk_d, wv_d, wo_d,
                     bq_d, bk_d, bv_d, bo_d, lnw_d, lnb_d, gidx_d, out_d,
                     nt, reps, trivial)
    nc.compile()
    return nc


def _kernel_body(nc, tc, q_d, k_d, v_d, wq_d, wk_d, wv_d, wo_d,
                 bq_d, bk_d, bv_d, bo_d, lnw_d, lnb_d, gidx_d, out_d,
                 nt=NT, reps=1, trivial=False):
    from contextlib import ExitStack
    ctx = ExitStack()
    with ctx:
        singles = ctx.enter_context(tc.tile_pool(name="singles", bufs=1))
        x_pool = ctx.enter_context(tc.tile_pool(name="x", bufs=2))
        ph_pool = ctx.enter_context(tc.tile_pool(name="ph", bufs=2))
        rg_pool = ctx.enter_context(tc.tile_pool(name="rg", bufs=2))
        sm_pool = ctx.enter_context(tc.tile_pool(name="sm", bufs=2))
        at_pool = ctx.enter_context(tc.tile_pool(name="at", bufs=2))
        y_pool = ctx.enter_context(tc.tile_pool(name="y", bufs=2))
        scr_pool = ctx.enter_context(tc.tile_pool(name="scr", bufs=1))
        psum_p = ctx.enter_context(tc.tile_pool(name="psp", bufs=2, space="PSUM"))
        psum_x = ctx.enter_context(tc.tile_pool(name="psx", bufs=2, space="PSUM"))
        psum_a = ctx.enter_context(tc.tile_pool(name="psa", bufs=2, space="PSUM"))
        psum_nt = ctx.enter_context(tc.tile_pool(name="psnt", bufs=2, space="PSUM"))

        # ---- persistent constants ----
        # (x-tile DMAs for the first pair are issued first in stage_a; load
        # wq before wk/wv, and wo last since o-proj runs 3 stages later)
        w_sb = {}
        for name, wd in (("q", wq_d), ("k", wk_d), ("v", wv_d), ("o", wo_d)):
            wt = singles.tile([P, KC, HID], BF16, tag=f"w{name}")
            w_sb[name] = wt

        def load_weights():
            for name, wd in (("q", wq_d), ("k", wk_d), ("v", wv_d)):
                nc.sync.dma_start(w_sb[name][:], wd[:])
            nc.scalar.dma_start(w_sb["o"][:], wo_d[:])

        gidx = singles.tile([P, 8], I16, tag="gidx")
        nc.scalar.dma_start(gidx[:], gidx_d[:])
        identb = singles.tile([P, P], BF16, tag="identb")
        make_identity(nc, identb)

        if not trivial:
            b_sb = {}
            for name, bd in (("q", bq_d), ("k", bk_d), ("v", bv_d)):
                t = singles.tile([P, KC], F32, tag=f"b{name}")
                nc.sync.dma_start(t[:], bd[:])
                b_sb[name] = t
            bo_sb = singles.tile([P, HID], BF16, tag="bo")
            nc.sync.dma_start(bo_sb[:], _bcast_dram(bo_d))
            lnw_sb = singles.tile([P, HID], BF16, tag="lnw")
            nc.sync.dma_start(lnw_sb[:], _bcast_dram(lnw_d))
            lnb_sb = singles.tile([P, HID], BF16, tag="lnb")
            nc.sync.dma_start(lnb_sb[:], _bcast_dram(lnb_d))

        state = {}
        preloaded = {}

        def emit_x_pair(it):
            xs = {}
            for name, xd in (("q", q_d), ("k", k_d), ("v", v_d)):
                xt = x_pool.tile([P, KC, 2 * P], BF16, tag=f"x{name}")
                lo = it * P
                hi = min(lo + 2 * P, nt * P)
                nc.sync.dma_start(xt[:, :, 0:hi - lo], xd[:, :, lo:hi])
                xs[name] = xt
            return xs

        def stage_a(pos, it):
            """Load x pair + q/k/v projections -> qhT/khT/vhT chunk layout.

            Projections run at pair granularity (rhs streams 256 tokens per
            128-col weight load) so PE weight loads stay hidden."""
            st = state[pos] = {}
            if pos % 2 == 0:
                st["x"] = preloaded.pop(pos, None) or emit_x_pair(it)
                nxt = {}
                for name in ("q", "k", "v"):
                    xt = st["x"][name]
                    phT0 = ph_pool.tile([P, KC, P], BF16, tag=f"phT{name}")
                    phT1 = ph_pool.tile([P, KC, P], BF16, tag=f"phT{name}")
                    for b in range(4):
                        pm = psum_p.tile([P, 512], F32, tag="proj")
                        for j in range(2):
                            oc = 2 * b + j
                            for c in range(KC):
                                nc.tensor.matmul(
                                    pm[:, j * 256:(j + 1) * 256],
                                    lhsT=w_sb[name][:, c, oc * P:(oc + 1) * P],
                                    rhs=xt[:, c, 0:2 * P],
                                    start=(c == 0), stop=(c == KC - 1))
                        for th, dst in ((0, phT0), (1, phT1)):
                            if trivial:
                                s_ap = bass.AP(
                                    tensor=pm.tensor,
                                    offset=pm.offset + th * P,
                                    ap=[[pm.ap[0][0], P], [256, 2], [1, P]])
                                nc.scalar.activation(
                                    out=dst[:, 2 * b:2 * b + 2, :].rearrange(
                                        "p a b -> p (a b)"),
                                    in_=s_ap, func=ACTF.Copy)
                            else:
                                for j in range(2):
                                    oc = 2 * b + j
                                    nc.scalar.activation(
                                        out=dst[:, oc, :],
                                        in_=pm[:, j * 256 + th * P:
                                               j * 256 + th * P + P],
                                        func=ACTF.Identity,
                                        bias=b_sb[name][:, oc:oc + 1])
                    st[f"phT{name}"] = phT0
                    nxt[f"phT{name}"] = phT1
                st["_nxt"] = nxt
            else:
                prev = state[pos - 1]
                st["x"] = prev["x"]
                for key in ("phTq", "phTk", "phTv"):
                    st[key] = prev["_nxt"][key]

            # qS/kS [64 d, h*128+s] via 2 DMAs each (hl partition halves)
            qS = rg_pool.tile([64, 16 * P], BF16, tag="qS")
            kS = rg_pool.tile([64, 16 * P], BF16, tag="kS")
            for dst, src, eng in ((qS, st["phTq"], nc.sync),
                                  (kS, st["phTk"], nc.scalar)):
                for hl in range(2):
                    s_ap = src[hl * 64:(hl + 1) * 64, :, :]
                    d_ap = bass.AP(tensor=dst.tensor, offset=dst.offset + hl * P,
                                   ap=[[dst.ap[0][0], 64], [256, KC], [1, P]])
                    eng.dma_start(d_ap, s_ap)
            st["qS"], st["kS"] = qS, kS

        def stage_b1(pos):
            """Cross matmuls + gather + softmax + vS regroup for tile at `pos`."""
            st = state[pos]
            qS, kS, vhT = st["qS"], st["kS"], st["phTv"]

            # shuffle to group-contiguous layout [64, g*128 + si*16 + h]
            # (matmul operands allow only one free dim)
            qS2 = rg_pool.tile([64, 16 * P], BF16, tag="qS2")
            kS2 = rg_pool.tile([64, 16 * P], BF16, tag="kS2")
            for dst, src in ((qS2, qS), (kS2, kS)):
                s_ap = bass.AP(tensor=src.tensor, offset=src.offset,
                               ap=[[src.ap[0][0], 64], [8, 16], [1, 8], [P, 16]])
                nc.vector.tensor_copy(out=dst[:], in_=s_ap)

            # cross matmuls: 16 groups of 8 tokens; quarter-psum [128, 512];
            # exp applied during PSUM->SBUF eviction (ACT), then one GPSIMD
            # gather extracts the diagonal blocks (GPSIMD can't read PSUM).
            crossE = scr_pool.tile([P, 4, 512], BF16, tag="crossE")
            for quarter in range(4):
                pt = psum_x.tile([P, 512], F32, tag="cross")
                for gl in range(4):
                    g = quarter * 4 + gl
                    nc.tensor.matmul(pt[:, gl * P:(gl + 1) * P],
                                     lhsT=qS2[:, g * P:(g + 1) * P],
                                     rhs=kS2[:, g * P:(g + 1) * P],
                                     start=True, stop=True)
                nc.scalar.activation(out=crossE[:, quarter, :], in_=pt[:],
                                     func=ACTF.Exp)

            # gather exp'd diagonal: wgtE [128 (si,h), (g,t)]
            wgtE = sm_pool.tile([P, 256], BF16, tag="wgtE")
            cflat = crossE.rearrange("p a b -> p (a b)")
            nc.gpsimd.ap_gather(
                out_ap=wgtE.rearrange("p (j d) -> p j d", d=2),
                in_ap=cflat.rearrange("p (j d) -> p j d", d=2),
                idxs_ap=gidx[:], channels=P, num_elems=1024, d=2, num_idxs=128)
            ssum = sm_pool.tile([P, 16], F32, tag="ssum")
            nc.vector.tensor_reduce(
                out=ssum[:], in_=wgtE.rearrange("p (g t) -> p g t", g=16),
                axis=AX.X, op=OP.add)
            rs = sm_pool.tile([P, 16], F32, tag="rs")
            nc.vector.reciprocal(out=rs[:], in_=ssum[:])
            wgt = sm_pool.tile([P, 16, 16], BF16, tag="wgt")
            nc.vector.tensor_tensor(
                out=wgt[:], in0=wgtE.rearrange("p (g t) -> p g t", g=16),
                in1=rs[:, :, None].to_broadcast((P, 16, 16)), op=OP.mult)
            st["wgt"] = wgt

            # vS [16 t, d*128+s] via 8 DMAs (dl 16-partition slices)
            vS = rg_pool.tile([16, 64 * P], BF16, tag="vS")
            for dl in range(8):
                s_ap = vhT[dl * 16:(dl + 1) * 16, :, :]
                d_ap = bass.AP(tensor=vS.tensor, offset=vS.offset + dl * P,
                               ap=[[vS.ap[0][0], 16], [8 * P, KC], [1, P]])
                eng = nc.gpsimd if dl < 6 else nc.sync
                eng.dma_start(d_ap, s_ap)
            st["vS"] = vS

        def stage_b2a(pos):
            """wgt transpose, attn matmuls, attnT regroup."""
            st = state[pos]
            wgt, vS = st["wgt"], st["vS"]
            wgtf = wgt.rearrange("p g t -> p (g t)")
            # per-g transposes -> wTT [16 t, g, (si,h)]
            wTT = at_pool.tile([16, 16, P], BF16, tag="wTT")
            pw = psum_nt.tile([P, 8 * P], BF16, tag="nt")
            for g in range(8):
                nc.tensor.transpose(pw[0:16, g * P:(g + 1) * P],
                                    wgtf[:, g * 16:(g + 1) * 16], identb)
            nc.vector.tensor_copy(
                out=wTT[:, 0:8, :].rearrange("p a b -> p (a b)"), in_=pw[0:16, :])
            pw2 = psum_nt.tile([P, 8 * P], BF16, tag="nt")
            for g in range(8):
                nc.tensor.transpose(pw2[0:16, g * P:(g + 1) * P],
                                    wgtf[:, (g + 8) * 16:(g + 9) * 16], identb)
            nc.vector.tensor_copy(
                out=wTT[:, 8:16, :].rearrange("p a b -> p (a b)"), in_=pw2[0:16, :])

            # per-token attn matmuls -> psum [64 d, (s,h)] quarters
            attnSB = at_pool.tile([64, 16, P], BF16, tag="attnSB")
            for quarter in range(4):
                pa = psum_a.tile([64, 512], F32, tag="attnp")
                for sl in range(32):
                    s = quarter * 32 + sl
                    g, si = s // 8, s % 8
                    lhsT = bass.AP(tensor=vS.tensor, offset=vS.offset + s,
                                   ap=[[vS.ap[0][0], 16], [P, 64]])
                    rhs = wTT[:, g, si * 16:(si + 1) * 16]
                    nc.tensor.matmul(pa[:, sl * 16:(sl + 1) * 16], lhsT=lhsT,
                                     rhs=rhs, start=True, stop=True)
                # evict rearranged (sl,h) -> cols h*128 + (quarter*32+sl)
                d_ap = bass.AP(tensor=attnSB.tensor,
                               offset=attnSB.offset + quarter * 32,
                               ap=[[attnSB.ap[0][0], 64], [1, 32], [P, 16]])
                nc.vector.tensor_copy(out=d_ap, in_=pa[:])

            # attnT [feat=(h,d), s] chunk layout via 2 regroup DMAs
            # (mirror of the qS regroup: attnSB is [64 d, h*128+s])
            attnT = at_pool.tile([P, KC, P], BF16, tag="attnT")
            for hl in range(2):
                s_ap = bass.AP(tensor=attnSB.tensor,
                               offset=attnSB.offset + hl * P,
                               ap=[[attnSB.ap[0][0], 64], [256, KC], [1, P]])
                d_ap = attnT[hl * 64:(hl + 1) * 64, :, :]
                nc.sync.dma_start(d_ap, s_ap)
            st["attnT"] = attnT

        def stage_b2b(pos, it):
            """o-proj + LN + store for tile `pos`."""
            st = state[pos]
            attnT = st["attnT"]
            # o-projection: y[s, n] in 2 halves + LN stats via ACT accumulate
            y = y_pool.tile([P, HID], BF16, tag="y")
            ysum = y_pool.tile([P, 2], F32, tag="ysum")
            for half in range(2):
                pm = psum_p.tile([P, 512], F32, tag="proj")
                for c in range(KC):
                    nc.tensor.matmul(pm[:], lhsT=attnT[:, c, :],
                                     rhs=w_sb["o"][:, c, half * 512:(half + 1) * 512],
                                     start=(c == 0), stop=(c == KC - 1))
                if trivial:
                    nc.scalar.activation(
                        out=y[:, half * 512:(half + 1) * 512], in_=pm[:],
                        func=ACTF.Copy, accum_out=ysum[:, half:half + 1])
                else:
                    nc.vector.tensor_tensor(
                        out=y[:, half * 512:(half + 1) * 512], in0=pm[:],
                        in1=bo_sb[:, half * 512:(half + 1) * 512], op=OP.add)
                    nc.scalar.activation(
                        out=y[:, half * 512:(half + 1) * 512],
                        in_=y[:, half * 512:(half + 1) * 512],
                        func=ACTF.Copy, accum_out=ysum[:, half:half + 1])

            # LN-ish: yout = ln_w*(y-mu)/(var_ddof1+eps) + ln_b
            ysq2 = y_pool.tile([P, 2], F32, tag="ysq2")
            scr = scr_pool.tile([P, 512], BF16, tag="scr")
            for half in range(2):
                nc.scalar.activation(
                    out=scr[:], in_=y[:, half * 512:(half + 1) * 512],
                    func=ACTF.Square, accum_out=ysq2[:, half:half + 1])
            ysq = y_pool.tile([P, 1], F32, tag="ysq")
            nc.vector.tensor_tensor(out=ysq[:], in0=ysq2[:, 0:1],
                                    in1=ysq2[:, 1:2], op=OP.add)
            s1 = y_pool.tile([P, 1], F32, tag="s1")
            nc.vector.tensor_tensor(out=s1[:], in0=ysum[:, 0:1],
                                    in1=ysum[:, 1:2], op=OP.add)
            mu = y_pool.tile([P, 1], F32, tag="mu")
            nc.vector.tensor_scalar(out=mu[:], in0=s1[:], scalar1=1.0 / HID,
                                    scalar2=None, op0=OP.mult)
            sq = y_pool.tile([P, 1], F32, tag="sq")
            nc.vector.tensor_tensor(out=sq[:], in0=s1[:], in1=mu[:], op=OP.mult)
            varr = y_pool.tile([P, 1], F32, tag="varr")
            nc.vector.tensor_scalar(out=varr[:], in0=ysq[:], scalar1=sq[:],
                                    scalar2=1.0 / (HID - 1), op0=OP.subtract,
                                    op1=OP.mult)
            nc.vector.tensor_scalar(out=varr[:], in0=varr[:], scalar1=EPS,
                                    scalar2=None, op0=OP.add)
            rv = y_pool.tile([P, 1], F32, tag="rv")
            nc.vector.reciprocal(out=rv[:], in_=varr[:])
            # yout pair tile: even pos allocates, odd pos fills 2nd half,
            # one DMA per pair
            if pos % 2 == 0:
                youtp = y_pool.tile([P, 2, HID], BF16, tag="yout")
                st["youtp"] = youtp
            else:
                youtp = state[pos - 1]["youtp"]
            yv = youtp[:, pos % 2, :]
            nc.vector.tensor_scalar(out=yv, in0=y[:], scalar1=mu[:],
                                    scalar2=rv[:], op0=OP.subtract, op1=OP.mult)
            if not trivial:
                nc.vector.tensor_tensor(out=yv, in0=yv, in1=lnw_sb[:],
                                        op=OP.mult)
                nc.vector.tensor_tensor(out=yv, in0=yv, in1=lnb_sb[:],
                                        op=OP.add)
            if pos % 2 == 1:
                it0 = it - 1
                d_ap = bass.AP(tensor=out_d.tensor,
                               offset=out_d.offset + it0 * P * HID,
                               ap=[[HID, P], [P * HID, 2], [1, HID]])
                nc.sync.dma_start(d_ap, youtp[:, 0:2, :])
                del state[pos - 1], state[pos]

        # software pipeline: B1(i-1) | A(i) | B2a(i-2) | B2b(i-3)
        tiles = [t for _ in range(reps) for t in range(nt)]
        n = len(tiles)
        preloaded[0] = emit_x_pair(tiles[0])
        load_weights()
        for i, it in enumerate(tiles):
            if i >= 1:
                stage_b1(i - 1)
            stage_a(i, it)
            if i >= 2:
                stage_b2a(i - 2)
            if i >= 3:
                stage_b2b(i - 3, tiles[i - 3])
        stage_b1(n - 1)
        for j in (n - 2, n - 1):
            if j >= 0:
                stage_b2a(j)
        for j in (n - 3, n - 2, n - 1):
            if j >= 0:
                stage_b2b(j, tiles[j])


def get_nc(nt=NT, reps=1, trivial=False):
    key = f"nc{nt}_{reps}_{trivial}"
    if key not in _CACHE:
        _CACHE[key] = _build_bass(nt, reps, trivial)
    return _CACHE[key]


def _prep_host(Wq, bq, Wk, bk, Wv, bv, Wo, bo, ln_w, ln_b):
    """Host-side prep: fold score scale into Wq/bq; permute Wv cols to
    d-major; cast to bf16 and rearrange [K,N] -> [ki, kc, n]."""
    def rearr(w):
        return np.ascontiguousarray(
            np.asarray(w, dtype=np.float32).reshape(KC, P, HID).transpose(1, 0, 2)
        ).astype(ml_dtypes.bfloat16)

    # v out-feature permutation: col t*64+d -> pos d*16+t
    perm_v = np.argsort(
        np.arange(HID).reshape(H, D).T.reshape(-1), kind="stable")
    perm_v = np.arange(HID).reshape(H, D).T.reshape(-1)  # pos (d,t) <- t*64+d
    Wv_p = np.asarray(Wv, np.float32)[:, perm_v]
    bv_p = np.asarray(bv, np.float32)[perm_v]

    def bias_arr(b):
        return np.ascontiguousarray(
            np.asarray(b, np.float32).reshape(KC, P).T).astype(np.float32)

    gidx = np.zeros((P, 8), np.int16)
    for si in range(8):
        vals = np.array([g * 64 + si * 8 + t2 for g in range(16)
                         for t2 in range(8)], np.int16)
        gidx[si * 16:(si + 1) * 16, :] = vals.reshape(8, 16).T

    return {
        "wq": rearr(np.asarray(Wq, np.float32) * SCALE),
        "wk": rearr(Wk),
        "wv": rearr(Wv_p),
        "wo": rearr(Wo),
        "bq": bias_arr(np.asarray(bq, np.float32) * np.float32(SCALE)),
        "bk": bias_arr(bk),
        "bv": bias_arr(bv_p),
        "bo": np.asarray(bo, np.float32).astype(ml_dtypes.bfloat16),
        "lnw": np.asarray(ln_w, np.float32).astype(ml_dtypes.bfloat16),
        "lnb": np.asarray(ln_b, np.float32).astype(ml_dtypes.bfloat16),
        "gidx": gidx,
    }


def _prep_x(x):
    """[T_CORE, HID] f32 -> [128 ki, 8 kc, T_CORE] bf16 (pre-transposed)."""
    xT = np.asarray(x, np.float32).T  # [HID, T_CORE]
    return np.ascontiguousarray(
        xT.reshape(KC, P, T_CORE).transpose(1, 0, 2)).astype(ml_dtypes.bfloat16)


def make_in_maps(q, k, v, Wq, bq, Wk, bk, Wv, bv, Wo, bo, ln_w, ln_b):
    shared = _prep_host(Wq, bq, Wk, bk, Wv, bv, Wo, bo, ln_w, ln_b)
    qf = np.asarray(q, np.float32).reshape(TOKENS, HID)
    kf = np.asarray(k, np.float32).reshape(TOKENS, HID)
    vf = np.asarray(v, np.float32).reshape(TOKENS, HID)
    in_maps = []
    for c in range(NCORES):
        sl = slice(c * T_CORE, (c + 1) * T_CORE)
        m = dict(shared)
        m["q"] = _prep_x(qf[sl])
        m["k"] = _prep_x(kf[sl])
        m["v"] = _prep_x(vf[sl])
        in_maps.append(m)
    return in_maps


def kernel(q, k, v, Wq, bq, Wk, bk, Wv, bv, Wo, bo, ln_w, ln_b):
    trivial = (not np.any(np.asarray(bq)) and not np.any(np.asarray(bk))
               and not np.any(np.asarray(bv)) and not np.any(np.asarray(bo))
               and np.all(np.asarray(ln_w) == 1.0)
               and not np.any(np.asarray(ln_b)))
    nc = get_nc(trivial=trivial)
    in_maps = make_in_maps(q, k, v, Wq, bq, Wk, bk, Wv, bv, Wo, bo, ln_w, ln_b)
    res = run_bass_kernel_spmd(nc, in_maps, list(range(NCORES))).results
    out = np.concatenate([np.asarray(res[c]["out"]) for c in range(NCORES)],
                         axis=0)
    return out.reshape(B, S, HID).astype(np.float32)
